# revision 6
# baseline (speedup 1.0000x reference)
"""Masked multi-head attention on 8 NeuronCores (faithful torch raw-view semantics).

The reference reshapes (bs, sql, nh*edim) -> (bs, nh, sql, edim) as a RAW VIEW:
head h's length-1024 pseudo-sequence is built from x rows 128h..128h+127 (each
row contributes 8 pseudo-positions, one per 256-col block of the projection),
and output rows 128h..128h+128 depend only on head h. So the work splits into
32 independent (batch, head) pairs -> 4 per core, no cross-core reduction.

v2: NATURAL pseudo-position ordering (column u = s' = r*8 + cb, a stride-8
scatter at projection writeback) makes the causal mask block-triangular, so
only 36 of 64 score/PV 128x128 blocks per head are computed (the baseline's
permuted ordering made every block half-masked -> full 64). All attention
matmuls run in bf16 (1 cycle/row at 128-wide tiles). The in-block causal
triangle on diagonal blocks is injected INTO PSUM by one small matmul
(step-matrix @ shifted-NEG-diag) instead of a DVE mask add. exp runs on the
Act engine straight from PSUM in per-kb strips; softmax denominators come from
bf16 ones-matmuls accumulated per q-block; V is re-laid out k-major via PE
transposes. Q weights/bias pre-scaled by 1/16.
"""

import sys

sys.path.insert(0, "/opt/trn_rl_repo")

import ml_dtypes
import numpy as np

from concourse import bacc, mybir
from concourse.tile import TileContext
from concourse.bass_utils import run_bass_kernel_spmd

EDIM = 256
BS = 4
SQL = 1024
HPC = 4           # heads per core
NCORES = 8
FDT = mybir.dt.float32
BF = mybir.dt.bfloat16
NEG = -1.0e30

# strip kb covers q-blocks kb..7; OFF[kb] = col offset of strip kb in pt
OFF = [0]
for _kb in range(1, 8):
    OFF.append(OFF[-1] + (8 - _kb + 1) * 128)
# chunks of <=4 q-blocks per strip (PSUM bank = 512 fp32 cols)
CHUNKS = {kb: [list(range(kb, 8))[i:i + 4]
               for i in range(0, 8 - kb, 4)] for kb in range(8)}

_cache = {}


def _build():
    nc = bacc.Bacc(dynamic_dma_scratch_size=512)

    xt0 = nc.declare_dram_parameter("xt0", [128, 512], BF, isOutput=False)
    xt1 = nc.declare_dram_parameter("xt1", [128, 512], BF, isOutput=False)
    wqk0 = nc.declare_dram_parameter("wqk0", [128, 4096], BF, isOutput=False)
    wqk1 = nc.declare_dram_parameter("wqk1", [128, 4096], BF, isOutput=False)
    wv0 = nc.declare_dram_parameter("wv0", [128, 2048], BF, isOutput=False)
    wv1 = nc.declare_dram_parameter("wv1", [128, 2048], BF, isOutput=False)
    bqk = nc.declare_dram_parameter("bqk", [128, 32], FDT, isOutput=False)
    bvp = nc.declare_dram_parameter("bvp", [128, 16], FDT, isOutput=False)
    wot = nc.declare_dram_parameter("wot", [128, 4096], BF, isOutput=False)
    stepA = nc.declare_dram_parameter("stepA", [128, 128], BF, isOutput=False)
    negB = nc.declare_dram_parameter("negB", [128, 128], BF, isOutput=False)
    onesc = nc.declare_dram_parameter("onesc", [128, 128], BF, isOutput=False)
    idn = nc.declare_dram_parameter("idn", [128, 128], BF, isOutput=False)
    y = nc.declare_dram_parameter("y", [512, 256], FDT, isOutput=True)

    EXP = mybir.ActivationFunctionType.Exp
    IDF = mybir.ActivationFunctionType.Identity

    with TileContext(nc) as tc:
        with (
            tc.tile_pool(name="const", bufs=1) as cpool,
            tc.tile_pool(name="big", bufs=1) as bpool,
            tc.tile_pool(name="pt", bufs=2) as ptpool,
            tc.tile_pool(name="oh", bufs=4) as ohpool,
            tc.tile_pool(name="rc", bufs=4) as rcpool,
            tc.tile_pool(name="ys", bufs=2) as yspool,
            tc.tile_pool(name="ps_a", bufs=2, space="PSUM") as poolA,
        ):
            def load(pool, name, src, shape, dt=FDT, tag=None):
                t = pool.tile(shape, dt, tag=tag or name, name=name)
                nc.sync.dma_start(out=t[:, :], in_=src)
                return t

            def mm(out, lhsT, rhs, **kw):
                nc.tensor.matmul(out, lhsT, rhs, **kw)

            # ---- DMAs in consumption order ----
            xt_sb = [load(cpool, "xt0", xt0[:, :], [128, 512], dt=BF),
                     load(cpool, "xt1", xt1[:, :], [128, 512], dt=BF)]
            wqk_srcs = [wqk0, wqk1]
            wqk_sb = []
            for d in range(2):
                wqk_sb.append(bpool.tile([128, 4096], BF, tag=f"wqk{d}",
                                         name=f"wqk{d}"))
            for s in range(2):          # Q cols then K cols
                for d in range(2):
                    for ch in range(2):
                        c0 = s * 2048 + ch * 1024
                        nc.sync.dma_start(
                            out=wqk_sb[d][:, c0:c0 + 1024],
                            in_=wqk_srcs[d][:, c0:c0 + 1024],
                        )
                if s == 0:
                    bqk_sb = load(cpool, "bqk", bqk[:, :], [128, 32])
            stepA_sb = load(cpool, "stepA", stepA[:, :], [128, 128], dt=BF)
            negB_sb = load(cpool, "negB", negB[:, :], [128, 128], dt=BF)
            ones_sb = load(cpool, "onesc", onesc[:, :], [128, 128], dt=BF)
            idn_sb = load(cpool, "idn", idn[:, :], [128, 128], dt=BF)
            bvp_sb = load(cpool, "bvp", bvp[:, :], [128, 16])
            wv_srcs = [wv0, wv1]
            wv_sb = []
            for d in range(2):
                t = bpool.tile([128, 2048], BF, tag=f"wv{d}", name=f"wv{d}")
                for ch in range(2):
                    nc.sync.dma_start(
                        out=t[:, ch * 1024:(ch + 1) * 1024],
                        in_=wv_srcs[d][:, ch * 1024:(ch + 1) * 1024],
                    )
                wv_sb.append(t)
            wot_sb = bpool.tile([128, 4096], BF, tag="wot", name="wot")
            for ch in range(2):
                nc.sync.dma_start(
                    out=wot_sb[:, ch * 2048:(ch + 1) * 2048],
                    in_=wot[:, ch * 2048:(ch + 1) * 2048],
                )

            # d-major Q^T/K^T/V^T, NATURAL order: cols = hl*1024 + s'
            qt = [bpool.tile([128, 4096], BF, tag=f"qt{c}", name=f"qt{c}")
                  for c in range(2)]
            kt = [bpool.tile([128, 4096], BF, tag=f"kt{c}", name=f"kt{c}")
                  for c in range(2)]
            vt = [bpool.tile([128, 4096], BF, tag=f"vt{c}", name=f"vt{c}")
                  for c in range(2)]
            # V k-major per head: [128 k, kb*256 + c*128 + d]
            v_nat = [bpool.tile([128, 2048], BF, tag=f"vn{hl}", name=f"vn{hl}")
                     for hl in range(HPC)]

            # ---- P1: Q^T/K^T projections (all heads), natural scatter ----
            nbias = 0
            for s in range(2):
                dst = qt if s == 0 else kt
                for cb in range(8):
                    for c in range(2):
                        ps = poolA.tile([128, 512], FDT, tag="pa", name="proj")
                        for d in range(2):
                            mm(
                                ps[:, :],
                                wqk_sb[d][:, s * 2048 + cb * 256 + c * 128:
                                          s * 2048 + cb * 256 + c * 128 + 128],
                                xt_sb[d][:, :],
                                start=(d == 0), stop=(d == 1),
                            )
                        bi = s * 16 + cb * 2 + c
                        out_v = dst[c].rearrange(
                            "p (h r e) -> p h r e", h=4, r=128, e=8
                        )[:, :, :, cb]
                        in_v = ps.rearrange("p (h r) -> p h r", h=4)[:, :, :]
                        if nbias % 2 == 0:
                            nc.scalar.activation(
                                out_v, in_v, IDF, bias=bqk_sb[:, bi:bi + 1]
                            )
                        else:
                            nc.vector.tensor_scalar_add(
                                out=out_v, in0=in_v,
                                scalar1=bqk_sb[:, bi:bi + 1],
                            )
                        nbias += 1

            # ---- P2: V^T projection (all heads), natural scatter ----
            for cb in range(8):
                for c in range(2):
                    ps = poolA.tile([128, 512], FDT, tag="pa", name="vproj")
                    for d in range(2):
                        mm(
                            ps[:, :],
                            wv_sb[d][:, cb * 256 + c * 128:
                                     cb * 256 + c * 128 + 128],
                            xt_sb[d][:, :],
                            start=(d == 0), stop=(d == 1),
                        )
                    bi = cb * 2 + c
                    out_v = vt[c].rearrange(
                        "p (h r e) -> p h r e", h=4, r=128, e=8
                    )[:, :, :, cb]
                    in_v = ps.rearrange("p (h r) -> p h r", h=4)[:, :, :]
                    if nbias % 2 == 0:
                        nc.scalar.activation(
                            out_v, in_v, IDF, bias=bvp_sb[:, bi:bi + 1]
                        )
                    else:
                        nc.vector.tensor_scalar_add(
                            out=out_v, in0=in_v, scalar1=bvp_sb[:, bi:bi + 1]
                        )
                    nbias += 1

            # ---- P2b: V -> k-major via PE transposes ----
            with tc.tile_pool(name="ps_t", bufs=2, space="PSUM") as tpool:
                ncp = 0
                for hl in range(HPC):
                    for kb in range(8):
                        tp = tpool.tile([128, 256], BF, tag="tp", name="tp")
                        for c in range(2):
                            nc.tensor.transpose(
                                tp[:, c * 128:(c + 1) * 128],
                                vt[c][:, hl * 1024 + kb * 128:
                                      hl * 1024 + (kb + 1) * 128],
                                idn_sb[:, :],
                            )
                        dst = v_nat[hl][:, kb * 256:(kb + 1) * 256]
                        if ncp % 2 == 0:
                            nc.scalar.copy(dst, tp[:, :])
                        else:
                            nc.vector.tensor_copy(out=dst, in_=tp[:, :])
                        ncp += 1

            # ---- P3: attention per head, causal block-skipped ----
            # pt strips per kb; PV/denominator as per-half [128,512] psum
            # accumulators with ONE wide matmul per (kb, half) — region qb
            # gets contributions from strips kb<=qb only, so per-mm stop
            # flags can't be exact: skip_group_check.
            with (
                tc.tile_pool(name="ps_se", bufs=2, space="PSUM") as pse,
                tc.tile_pool(name="ps_po", bufs=4, space="PSUM") as ppo,
            ):
                for hl in range(HPC):
                    qoff = hl * 1024
                    pt_h = ptpool.tile([128, 4608], BF, tag="pt", name="pt")
                    oh = [ohpool.tile([128, 1024], BF, tag="oh",
                                      name=f"oh{c}") for c in range(2)]

                    def strips(kbs, hl=hl, qoff=qoff, pt_h=pt_h):
                        for kb in kbs:
                            for chunk in CHUNKS[kb]:
                                n = len(chunk) * 128
                                sp = poolA.tile([128, 512], FDT, tag="pa",
                                                name="score")
                                if chunk[0] == kb:   # diagonal block first
                                    mm(sp[:, 0:128], stepA_sb[:, :],
                                       negB_sb[:, :], start=True, stop=False)
                                    for c in range(2):
                                        mm(sp[:, 0:128],
                                           kt[c][:, qoff + kb * 128:
                                                 qoff + kb * 128 + 128],
                                           qt[c][:, qoff + kb * 128:
                                                 qoff + kb * 128 + 128],
                                           start=False, stop=(c == 1))
                                    q0, w = 128, n - 128
                                else:
                                    q0, w = 0, n
                                if w > 0:
                                    qc0 = qoff + (chunk[0] * 128) + q0
                                    for c in range(2):
                                        mm(sp[:, q0:q0 + w],
                                           kt[c][:, qoff + kb * 128:
                                                 qoff + kb * 128 + 128],
                                           qt[c][:, qc0:qc0 + w],
                                           start=(c == 0), stop=(c == 1))
                                o0 = OFF[kb] + (chunk[0] - kb) * 128
                                nc.scalar.activation(
                                    pt_h[:, o0:o0 + n], sp[:, 0:n], EXP
                                )

                    def pv_half(half, hl=hl, pt_h=pt_h, oh=oh):
                        # cols [half*512, half*512+512) of the q axis
                        lo = half * 4
                        se = pse.tile([128, 512], FDT, tag="se", name="se")
                        pos = [ppo.tile([128, 512], FDT, tag="po",
                                        name=f"po{c}") for c in range(2)]
                        kmax = lo + 4
                        for c in range(-1, 2):
                            dst = se if c < 0 else pos[c]
                            for kb in range(kmax):
                                # q blocks covered: max(kb, lo)..lo+3
                                qb0 = max(kb, lo)
                                w = (lo + 4 - qb0) * 128
                                o = OFF[kb] + (qb0 - kb) * 128
                                r0 = (qb0 - lo) * 128
                                lhsT = ones_sb[:, :] if c < 0 else (
                                    v_nat[hl][:, kb * 256 + c * 128:
                                              kb * 256 + c * 128 + 128])
                                mm(dst[:, r0:r0 + w], lhsT,
                                   pt_h[:, o:o + w],
                                   start=(kb == 0), stop=(kb == kmax - 1),
                                   skip_group_check=True)
                        rc = rcpool.tile([128, 512], FDT, tag="rc", name="rc")
                        nc.vector.reciprocal(out=rc[:, :], in_=se[:, :])
                        for c in range(2):
                            nc.vector.tensor_mul(
                                out=oh[c][:, half * 512:half * 512 + 512],
                                in0=pos[c][:, :], in1=rc[:, :],
                            )

                    strips(range(0, 4))
                    pv_half(0)
                    strips(range(4, 8))
                    pv_half(1)

                    # ---- P4: output projection for this head ----
                    yp = pse.tile([128, 512], FDT, tag="se", name="yproj")
                    for cb in range(8):
                        for c in range(2):
                            j = cb * 2 + c
                            mm(
                                yp[:, 0:256],
                                oh[c].rearrange(
                                    "p (r e) -> p r e", r=128, e=8
                                )[:, :, cb],
                                wot_sb[:, j * 256:(j + 1) * 256],
                                start=(j == 0), stop=(j == 15),
                            )
                    ys = yspool.tile([128, 256], FDT, tag="ys", name="ys")
                    nc.scalar.copy(ys[:, :], yp[:, 0:256])
                    nc.sync.dma_start(
                        out=y[hl * 128:(hl + 1) * 128, :], in_=ys[:, :]
                    )
    nc.finalize()
    return nc


def _prep_inputs(x, w_attn, b_attn, w_out):
    b16 = ml_dtypes.bfloat16
    wqk = np.ascontiguousarray(
        np.concatenate([w_attn[0:2048] / 16.0, w_attn[2048:4096]]).T
    ).astype(b16)  # (256, 4096)
    wvt = np.ascontiguousarray(w_attn[4096:6144].T).astype(b16)  # (256, 2048)
    bqk_arr = np.ascontiguousarray(
        np.concatenate([b_attn[0:2048] / 16.0, b_attn[2048:4096]])
        .reshape(32, 128).T
    ).astype(np.float32)  # (128, 32)
    bvp_arr = np.ascontiguousarray(
        b_attn[4096:6144].reshape(16, 128).T
    ).astype(np.float32)  # (128, 16)
    wot_arr = np.ascontiguousarray(
        w_out.T.reshape(16, 128, 256).transpose(1, 0, 2).reshape(128, 4096)
    ).astype(b16)

    stepA = np.triu(np.ones((128, 128), np.float32)).astype(b16)
    negB = (NEG * np.eye(128, k=-1)).astype(b16)
    onesc = np.ones((128, 128), np.float32).astype(b16)
    idn = np.eye(128, dtype=np.float32).astype(b16)

    in_maps = []
    for cidx in range(NCORES):
        b, g = divmod(cidx, 2)
        xt = np.ascontiguousarray(
            x[b, 512 * g:512 * (g + 1)].T
        ).astype(b16)  # (256, 512)
        in_maps.append({
            "xt0": np.ascontiguousarray(xt[:128]),
            "xt1": np.ascontiguousarray(xt[128:]),
            "wqk0": np.ascontiguousarray(wqk[:128]),
            "wqk1": np.ascontiguousarray(wqk[128:]),
            "wv0": np.ascontiguousarray(wvt[:128]),
            "wv1": np.ascontiguousarray(wvt[128:]),
            "bqk": bqk_arr,
            "bvp": bvp_arr,
            "wot": wot_arr,
            "stepA": stepA,
            "negB": negB,
            "onesc": onesc,
            "idn": idn,
        })
    return in_maps


def kernel(x, w_attn, b_attn, w_out, b_out):
    x = np.asarray(x, dtype=np.float32)
    w_attn = np.asarray(w_attn, dtype=np.float32)
    b_attn = np.asarray(b_attn, dtype=np.float32)
    w_out = np.asarray(w_out, dtype=np.float32)
    b_out = np.asarray(b_out, dtype=np.float32)

    if "nc" not in _cache:
        _cache["nc"] = _build()
    nc = _cache["nc"]

    in_maps = _prep_inputs(x, w_attn, b_attn, w_out)
    res = run_bass_kernel_spmd(nc, in_maps, list(range(NCORES))).results

    out = np.empty((BS, SQL, EDIM), dtype=np.float32)
    for c in range(NCORES):
        b, g = divmod(c, 2)
        out[b, 512 * g:512 * (g + 1)] = res[c]["y"]
    out += b_out
    return out


# revision 10
# speedup vs baseline: 1.1410x; 1.1410x over previous
"""Masked multi-head attention on 8 NeuronCores (faithful torch raw-view semantics).

The reference reshapes (bs, sql, nh*edim) -> (bs, nh, sql, edim) as a RAW VIEW:
head h's length-1024 pseudo-sequence is built from x rows 128h..128h+127 (each
row contributes 8 pseudo-positions, one per 256-col block of the projection),
and output rows 128h..128h+128 depend only on head h. So the work splits into
32 independent (batch, head) pairs -> 4 per core, no cross-core reduction.

v2: NATURAL pseudo-position ordering (column u = s' = r*8 + cb, a stride-8
scatter at projection writeback) makes the causal mask block-triangular, so
only 36 of 64 score/PV 128x128 blocks per head are computed (the baseline's
permuted ordering made every block half-masked -> full 64). All attention
matmuls run in bf16 (1 cycle/row at 128-wide tiles). The in-block causal
triangle on diagonal blocks is injected INTO PSUM by one small matmul
(step-matrix @ shifted-NEG-diag) instead of a DVE mask add. exp runs on the
Act engine straight from PSUM in per-kb strips; softmax denominators come from
bf16 ones-matmuls accumulated per q-block; V is re-laid out k-major via PE
transposes. Q weights/bias pre-scaled by 1/16.
"""

import sys

sys.path.insert(0, "/opt/trn_rl_repo")

import ml_dtypes
import numpy as np

from concourse import bacc, mybir
from concourse.tile import TileContext
from concourse.bass_utils import run_bass_kernel_spmd

EDIM = 256
BS = 4
SQL = 1024
HPC = 4           # heads per core
NCORES = 8
FDT = mybir.dt.float32
BF = mybir.dt.bfloat16
NEG = -1.0e30

# strip kb covers q-blocks kb..7; OFF[kb] = col offset of strip kb in pt
OFF = [0]
for _kb in range(1, 8):
    OFF.append(OFF[-1] + (8 - _kb + 1) * 128)
# chunks of <=4 q-blocks per strip (PSUM bank = 512 fp32 cols)
CHUNKS = {kb: [list(range(kb, 8))[i:i + 4]
               for i in range(0, 8 - kb, 4)] for kb in range(8)}

_cache = {}


def _build():
    nc = bacc.Bacc(dynamic_dma_scratch_size=512)

    xt0 = nc.declare_dram_parameter("xt0", [128, 512], BF, isOutput=False)
    xt1 = nc.declare_dram_parameter("xt1", [128, 512], BF, isOutput=False)
    wqk0 = nc.declare_dram_parameter("wqk0", [128, 4096], BF, isOutput=False)
    wqk1 = nc.declare_dram_parameter("wqk1", [128, 4096], BF, isOutput=False)
    wv0 = nc.declare_dram_parameter("wv0", [128, 2048], BF, isOutput=False)
    wv1 = nc.declare_dram_parameter("wv1", [128, 2048], BF, isOutput=False)
    bqk = nc.declare_dram_parameter("bqk", [128, 32], FDT, isOutput=False)
    bvp = nc.declare_dram_parameter("bvp", [128, 16], FDT, isOutput=False)
    wot = nc.declare_dram_parameter("wot", [128, 4096], BF, isOutput=False)
    stepA = nc.declare_dram_parameter("stepA", [128, 128], BF, isOutput=False)
    negB = nc.declare_dram_parameter("negB", [128, 128], BF, isOutput=False)
    onesc = nc.declare_dram_parameter("onesc", [128, 128], BF, isOutput=False)
    idn = nc.declare_dram_parameter("idn", [128, 128], BF, isOutput=False)
    y = nc.declare_dram_parameter("y", [512, 256], FDT, isOutput=True)

    EXP = mybir.ActivationFunctionType.Exp
    IDF = mybir.ActivationFunctionType.Identity

    with TileContext(nc) as tc:
        with (
            tc.tile_pool(name="const", bufs=1) as cpool,
            tc.tile_pool(name="big", bufs=1) as bpool,
            tc.tile_pool(name="pt", bufs=2) as ptpool,
            tc.tile_pool(name="oh", bufs=4) as ohpool,
            tc.tile_pool(name="rc", bufs=4) as rcpool,
            tc.tile_pool(name="ys", bufs=2) as yspool,
            tc.tile_pool(name="ps_a", bufs=2, space="PSUM") as poolA,
        ):
            def load(pool, name, src, shape, dt=FDT, tag=None):
                t = pool.tile(shape, dt, tag=tag or name, name=name)
                nc.sync.dma_start(out=t[:, :], in_=src)
                return t

            def mm(out, lhsT, rhs, **kw):
                nc.tensor.matmul(out, lhsT, rhs, **kw)

            # ---- DMAs in consumption order ----
            # xt on the Activation HWDGE queue, weights on SP: parallel issue
            xt_sb = []
            for i, src in enumerate((xt0, xt1)):
                t = cpool.tile([128, 512], BF, tag=f"xt{i}", name=f"xt{i}")
                nc.scalar.dma_start(out=t[:, :], in_=src[:, :])
                xt_sb.append(t)
            wqk_srcs = [wqk0, wqk1]
            wqk_sb = []
            for d in range(2):
                wqk_sb.append(bpool.tile([128, 4096], BF, tag=f"wqk{d}",
                                         name=f"wqk{d}"))
            for s in range(2):          # Q cols then K cols
                for d in range(2):
                    for ch in range(2):
                        c0 = s * 2048 + ch * 1024
                        nc.sync.dma_start(
                            out=wqk_sb[d][:, c0:c0 + 1024],
                            in_=wqk_srcs[d][:, c0:c0 + 1024],
                        )
                if s == 0:
                    bqk_sb = load(cpool, "bqk", bqk[:, :], [128, 32])
            stepA_sb = load(cpool, "stepA", stepA[:, :], [128, 128], dt=BF)
            negB_sb = load(cpool, "negB", negB[:, :], [128, 128], dt=BF)
            ones_sb = load(cpool, "onesc", onesc[:, :], [128, 128], dt=BF)
            idn_sb = load(cpool, "idn", idn[:, :], [128, 128], dt=BF)
            bvp_sb = load(cpool, "bvp", bvp[:, :], [128, 16])
            wv_srcs = [wv0, wv1]
            wv_sb = []
            for d in range(2):
                t = bpool.tile([128, 2048], BF, tag=f"wv{d}", name=f"wv{d}")
                for ch in range(2):
                    nc.sync.dma_start(
                        out=t[:, ch * 1024:(ch + 1) * 1024],
                        in_=wv_srcs[d][:, ch * 1024:(ch + 1) * 1024],
                    )
                wv_sb.append(t)
            wot_sb = bpool.tile([128, 4096], BF, tag="wot", name="wot")
            for ch in range(2):
                nc.sync.dma_start(
                    out=wot_sb[:, ch * 2048:(ch + 1) * 2048],
                    in_=wot[:, ch * 2048:(ch + 1) * 2048],
                )

            # d-major Q^T/K^T/V^T, NATURAL order: cols = hl*1024 + s'
            qt = [bpool.tile([128, 4096], BF, tag=f"qt{c}", name=f"qt{c}")
                  for c in range(2)]
            kt = [bpool.tile([128, 4096], BF, tag=f"kt{c}", name=f"kt{c}")
                  for c in range(2)]
            vt = [bpool.tile([128, 4096], BF, tag=f"vt{c}", name=f"vt{c}")
                  for c in range(2)]
            # V k-major per head: [128 k, kb*256 + c*128 + d]
            v_nat = [bpool.tile([128, 2048], BF, tag=f"vn{hl}", name=f"vn{hl}")
                     for hl in range(HPC)]

            # ---- P1: Q^T/K^T projections (all heads), natural scatter ----
            pj_ctx = tc.tile_pool(name="ps_pj", bufs=4, space="PSUM")
            pjpool = pj_ctx.__enter__()
            nbias = 0
            for s in range(2):
                dst = qt if s == 0 else kt
                for cb in range(8):
                    for c in range(2):
                        ps = pjpool.tile([128, 512], FDT, tag="pj",
                                         name="proj")
                        for d in range(2):
                            mm(
                                ps[:, :],
                                wqk_sb[d][:, s * 2048 + cb * 256 + c * 128:
                                          s * 2048 + cb * 256 + c * 128 + 128],
                                xt_sb[d][:, :],
                                start=(d == 0), stop=(d == 1),
                            )
                        bi = s * 16 + cb * 2 + c
                        out_v = dst[c].rearrange(
                            "p (h r e) -> p h r e", h=4, r=128, e=8
                        )[:, :, :, cb]
                        in_v = ps.rearrange("p (h r) -> p h r", h=4)[:, :, :]
                        if nbias % 2 == 0:
                            nc.scalar.activation(
                                out_v, in_v, IDF, bias=bqk_sb[:, bi:bi + 1]
                            )
                        else:
                            nc.vector.tensor_scalar_add(
                                out=out_v, in0=in_v,
                                scalar1=bqk_sb[:, bi:bi + 1],
                            )
                        nbias += 1

            # ---- P2: V^T projection (all heads), natural scatter ----
            for cb in range(8):
                for c in range(2):
                    ps = pjpool.tile([128, 512], FDT, tag="pj", name="vproj")
                    for d in range(2):
                        mm(
                            ps[:, :],
                            wv_sb[d][:, cb * 256 + c * 128:
                                     cb * 256 + c * 128 + 128],
                            xt_sb[d][:, :],
                            start=(d == 0), stop=(d == 1),
                        )
                    bi = cb * 2 + c
                    out_v = vt[c].rearrange(
                        "p (h r e) -> p h r e", h=4, r=128, e=8
                    )[:, :, :, cb]
                    in_v = ps.rearrange("p (h r) -> p h r", h=4)[:, :, :]
                    if nbias % 2 == 0:
                        nc.scalar.activation(
                            out_v, in_v, IDF, bias=bvp_sb[:, bi:bi + 1]
                        )
                    else:
                        nc.vector.tensor_scalar_add(
                            out=out_v, in0=in_v, scalar1=bvp_sb[:, bi:bi + 1]
                        )
                    nbias += 1
            pj_ctx.__exit__(None, None, None)

            # ---- P2b: V -> k-major via PE transposes ----
            with tc.tile_pool(name="ps_t", bufs=2, space="PSUM") as tpool:
                ncp = 0
                for hl in range(HPC):
                    for kb in range(8):
                        tp = tpool.tile([128, 256], BF, tag="tp", name="tp")
                        for c in range(2):
                            nc.tensor.transpose(
                                tp[:, c * 128:(c + 1) * 128],
                                vt[c][:, hl * 1024 + kb * 128:
                                      hl * 1024 + (kb + 1) * 128],
                                idn_sb[:, :],
                            )
                        dst = v_nat[hl][:, kb * 256:(kb + 1) * 256]
                        if ncp % 2 == 0:
                            nc.scalar.copy(dst, tp[:, :])
                        else:
                            nc.vector.tensor_copy(out=dst, in_=tp[:, :])
                        ncp += 1

            # ---- P3: attention per head, causal block-skipped ----
            # pt strips per kb; PV/denominator as per-half [128,512] psum
            # accumulators with ONE wide matmul per (kb, half) — region qb
            # gets contributions from strips kb<=qb only, so per-mm stop
            # flags can't be exact: skip_group_check.
            with (
                tc.tile_pool(name="ps_se", bufs=2, space="PSUM") as pse,
                tc.tile_pool(name="ps_po", bufs=4, space="PSUM") as ppo,
            ):
                for hl in range(HPC):
                    qoff = hl * 1024
                    pt_h = ptpool.tile([128, 4608], BF, tag="pt", name="pt")
                    oh = [ohpool.tile([128, 1024], BF, tag="oh",
                                      name=f"oh{c}") for c in range(2)]

                    def strips(kbs, hl=hl, qoff=qoff, pt_h=pt_h):
                        for kb in kbs:
                            for chunk in CHUNKS[kb]:
                                n = len(chunk) * 128
                                sp = poolA.tile([128, 512], FDT, tag="pa",
                                                name="score")
                                if chunk[0] == kb:   # diagonal block first
                                    mm(sp[:, 0:128], stepA_sb[:, :],
                                       negB_sb[:, :], start=True, stop=False)
                                    for c in range(2):
                                        mm(sp[:, 0:128],
                                           kt[c][:, qoff + kb * 128:
                                                 qoff + kb * 128 + 128],
                                           qt[c][:, qoff + kb * 128:
                                                 qoff + kb * 128 + 128],
                                           start=False, stop=(c == 1))
                                    q0, w = 128, n - 128
                                else:
                                    q0, w = 0, n
                                if w > 0:
                                    qc0 = qoff + (chunk[0] * 128) + q0
                                    for c in range(2):
                                        mm(sp[:, q0:q0 + w],
                                           kt[c][:, qoff + kb * 128:
                                                 qoff + kb * 128 + 128],
                                           qt[c][:, qc0:qc0 + w],
                                           start=(c == 0), stop=(c == 1))
                                o0 = OFF[kb] + (chunk[0] - kb) * 128
                                nc.scalar.activation(
                                    pt_h[:, o0:o0 + n], sp[:, 0:n], EXP
                                )

                    def pv_half(half, hl=hl, pt_h=pt_h, oh=oh):
                        # cols [half*512, half*512+512) of the q axis
                        lo = half * 4
                        se = pse.tile([128, 512], FDT, tag="se", name="se")
                        pos = [ppo.tile([128, 512], FDT, tag="po",
                                        name=f"po{c}") for c in range(2)]
                        kmax = lo + 4
                        for c in range(-1, 2):
                            dst = se if c < 0 else pos[c]
                            for kb in range(kmax):
                                # q blocks covered: max(kb, lo)..lo+3
                                qb0 = max(kb, lo)
                                w = (lo + 4 - qb0) * 128
                                o = OFF[kb] + (qb0 - kb) * 128
                                r0 = (qb0 - lo) * 128
                                lhsT = ones_sb[:, :] if c < 0 else (
                                    v_nat[hl][:, kb * 256 + c * 128:
                                              kb * 256 + c * 128 + 128])
                                mm(dst[:, r0:r0 + w], lhsT,
                                   pt_h[:, o:o + w],
                                   start=(kb == 0), stop=(kb == kmax - 1),
                                   skip_group_check=True)
                        rc = rcpool.tile([128, 512], FDT, tag="rc", name="rc")
                        nc.vector.reciprocal(out=rc[:, :], in_=se[:, :])
                        for c in range(2):
                            nc.vector.tensor_mul(
                                out=oh[c][:, half * 512:half * 512 + 512],
                                in0=pos[c][:, :], in1=rc[:, :],
                            )

                    strips(range(0, 4))
                    pv_half(0)
                    strips(range(4, 8))
                    pv_half(1)

                    # ---- P4: output projection for this head ----
                    yp = pse.tile([128, 512], FDT, tag="se", name="yproj")
                    for cb in range(8):
                        for c in range(2):
                            j = cb * 2 + c
                            mm(
                                yp[:, 0:256],
                                oh[c].rearrange(
                                    "p (r e) -> p r e", r=128, e=8
                                )[:, :, cb],
                                wot_sb[:, j * 256:(j + 1) * 256],
                                start=(j == 0), stop=(j == 15),
                            )
                    ys = yspool.tile([128, 256], FDT, tag="ys", name="ys")
                    nc.scalar.copy(ys[:, :], yp[:, 0:256])
                    nc.sync.dma_start(
                        out=y[hl * 128:(hl + 1) * 128, :], in_=ys[:, :]
                    )
    nc.finalize()
    return nc


def _prep_inputs(x, w_attn, b_attn, w_out):
    b16 = ml_dtypes.bfloat16
    wqk = np.ascontiguousarray(
        np.concatenate([w_attn[0:2048] / 16.0, w_attn[2048:4096]]).T
    ).astype(b16)  # (256, 4096)
    wvt = np.ascontiguousarray(w_attn[4096:6144].T).astype(b16)  # (256, 2048)
    bqk_arr = np.ascontiguousarray(
        np.concatenate([b_attn[0:2048] / 16.0, b_attn[2048:4096]])
        .reshape(32, 128).T
    ).astype(np.float32)  # (128, 32)
    bvp_arr = np.ascontiguousarray(
        b_attn[4096:6144].reshape(16, 128).T
    ).astype(np.float32)  # (128, 16)
    wot_arr = np.ascontiguousarray(
        w_out.T.reshape(16, 128, 256).transpose(1, 0, 2).reshape(128, 4096)
    ).astype(b16)

    stepA = np.triu(np.ones((128, 128), np.float32)).astype(b16)
    negB = (NEG * np.eye(128, k=-1)).astype(b16)
    onesc = np.ones((128, 128), np.float32).astype(b16)
    idn = np.eye(128, dtype=np.float32).astype(b16)

    in_maps = []
    for cidx in range(NCORES):
        b, g = divmod(cidx, 2)
        xt = np.ascontiguousarray(
            x[b, 512 * g:512 * (g + 1)].T
        ).astype(b16)  # (256, 512)
        in_maps.append({
            "xt0": np.ascontiguousarray(xt[:128]),
            "xt1": np.ascontiguousarray(xt[128:]),
            "wqk0": np.ascontiguousarray(wqk[:128]),
            "wqk1": np.ascontiguousarray(wqk[128:]),
            "wv0": np.ascontiguousarray(wvt[:128]),
            "wv1": np.ascontiguousarray(wvt[128:]),
            "bqk": bqk_arr,
            "bvp": bvp_arr,
            "wot": wot_arr,
            "stepA": stepA,
            "negB": negB,
            "onesc": onesc,
            "idn": idn,
        })
    return in_maps


def kernel(x, w_attn, b_attn, w_out, b_out):
    x = np.asarray(x, dtype=np.float32)
    w_attn = np.asarray(w_attn, dtype=np.float32)
    b_attn = np.asarray(b_attn, dtype=np.float32)
    w_out = np.asarray(w_out, dtype=np.float32)
    b_out = np.asarray(b_out, dtype=np.float32)

    if "nc" not in _cache:
        _cache["nc"] = _build()
    nc = _cache["nc"]

    in_maps = _prep_inputs(x, w_attn, b_attn, w_out)
    res = run_bass_kernel_spmd(nc, in_maps, list(range(NCORES))).results

    out = np.empty((BS, SQL, EDIM), dtype=np.float32)
    for c in range(NCORES):
        b, g = divmod(c, 2)
        out[b, 512 * g:512 * (g + 1)] = res[c]["y"]
    out += b_out
    return out


# revision 12
# speedup vs baseline: 1.1436x; 1.0023x over previous
"""Masked multi-head attention on 8 NeuronCores (faithful torch raw-view semantics).

The reference reshapes (bs, sql, nh*edim) -> (bs, nh, sql, edim) as a RAW VIEW:
head h's length-1024 pseudo-sequence is built from x rows 128h..128h+127 (each
row contributes 8 pseudo-positions, one per 256-col block of the projection),
and output rows 128h..128h+128 depend only on head h. So the work splits into
32 independent (batch, head) pairs -> 4 per core, no cross-core reduction.

v2: NATURAL pseudo-position ordering (column u = s' = r*8 + cb, a stride-8
scatter at projection writeback) makes the causal mask block-triangular, so
only 36 of 64 score/PV 128x128 blocks per head are computed (the baseline's
permuted ordering made every block half-masked -> full 64). All attention
matmuls run in bf16 (1 cycle/row at 128-wide tiles). The in-block causal
triangle on diagonal blocks is injected INTO PSUM by one small matmul
(step-matrix @ shifted-NEG-diag) instead of a DVE mask add. exp runs on the
Act engine straight from PSUM in per-kb strips; softmax denominators come from
bf16 ones-matmuls accumulated per q-block; V is re-laid out k-major via PE
transposes. Q weights/bias pre-scaled by 1/16.
"""

import sys

sys.path.insert(0, "/opt/trn_rl_repo")

import ml_dtypes
import numpy as np

from concourse import bacc, mybir
from concourse.tile import TileContext
from concourse.bass_utils import run_bass_kernel_spmd

EDIM = 256
BS = 4
SQL = 1024
HPC = 4           # heads per core
NCORES = 8
FDT = mybir.dt.float32
BF = mybir.dt.bfloat16
NEG = -1.0e30

# strip kb covers q-blocks kb..7; OFF[kb] = col offset of strip kb in pt
OFF = [0]
for _kb in range(1, 8):
    OFF.append(OFF[-1] + (8 - _kb + 1) * 128)
# chunks of <=4 q-blocks per strip (PSUM bank = 512 fp32 cols)
CHUNKS = {kb: [list(range(kb, 8))[i:i + 4]
               for i in range(0, 8 - kb, 4)] for kb in range(8)}

_cache = {}


def _build():
    nc = bacc.Bacc(dynamic_dma_scratch_size=512)

    xt0 = nc.declare_dram_parameter("xt0", [128, 512], BF, isOutput=False)
    xt1 = nc.declare_dram_parameter("xt1", [128, 512], BF, isOutput=False)
    wqk0 = nc.declare_dram_parameter("wqk0", [128, 4096], BF, isOutput=False)
    wqk1 = nc.declare_dram_parameter("wqk1", [128, 4096], BF, isOutput=False)
    wv0 = nc.declare_dram_parameter("wv0", [128, 2048], BF, isOutput=False)
    wv1 = nc.declare_dram_parameter("wv1", [128, 2048], BF, isOutput=False)
    bqk = nc.declare_dram_parameter("bqk", [128, 32], FDT, isOutput=False)
    bvp = nc.declare_dram_parameter("bvp", [128, 16], FDT, isOutput=False)
    wot = nc.declare_dram_parameter("wot", [128, 4096], BF, isOutput=False)
    stepA = nc.declare_dram_parameter("stepA", [128, 128], BF, isOutput=False)
    negB = nc.declare_dram_parameter("negB", [128, 128], BF, isOutput=False)
    onesc = nc.declare_dram_parameter("onesc", [128, 128], BF, isOutput=False)
    idn = nc.declare_dram_parameter("idn", [128, 128], BF, isOutput=False)
    y = nc.declare_dram_parameter("y", [512, 256], FDT, isOutput=True)

    EXP = mybir.ActivationFunctionType.Exp
    IDF = mybir.ActivationFunctionType.Identity

    with TileContext(nc) as tc:
        with (
            tc.tile_pool(name="const", bufs=1) as cpool,
            tc.tile_pool(name="big", bufs=1) as bpool,
            tc.tile_pool(name="pt", bufs=2) as ptpool,
            tc.tile_pool(name="oh", bufs=4) as ohpool,
            tc.tile_pool(name="rc", bufs=4) as rcpool,
            tc.tile_pool(name="ys", bufs=2) as yspool,
            tc.tile_pool(name="ps_a", bufs=2, space="PSUM") as poolA,
        ):
            def load(pool, name, src, shape, dt=FDT, tag=None):
                t = pool.tile(shape, dt, tag=tag or name, name=name)
                nc.sync.dma_start(out=t[:, :], in_=src)
                return t

            def mm(out, lhsT, rhs, **kw):
                nc.tensor.matmul(out, lhsT, rhs, **kw)

            # ---- PE warmup: keep PE busy from t=0 on a zeroed const so the
            # p-state ramp (3us to full clock) burns off before real work ----
            wz = cpool.tile([128, 512], BF, tag="wz", name="warmzero")
            nc.vector.memset(wz[:, :], 0.0)

            # ---- DMAs in consumption order ----
            # xt on the Activation HWDGE queue, weights on SP: parallel issue
            xt_sb = []
            for i, src in enumerate((xt0, xt1)):
                t = cpool.tile([128, 512], BF, tag=f"xt{i}", name=f"xt{i}")
                nc.scalar.dma_start(out=t[:, :], in_=src[:, :])
                xt_sb.append(t)
            wqk_srcs = [wqk0, wqk1]
            wqk_sb = []
            for d in range(2):
                wqk_sb.append(bpool.tile([128, 4096], BF, tag=f"wqk{d}",
                                         name=f"wqk{d}"))
            for s in range(2):          # Q cols then K cols
                for d in range(2):
                    for ch in range(2):
                        c0 = s * 2048 + ch * 1024
                        nc.sync.dma_start(
                            out=wqk_sb[d][:, c0:c0 + 1024],
                            in_=wqk_srcs[d][:, c0:c0 + 1024],
                        )
                if s == 0:
                    bqk_sb = load(cpool, "bqk", bqk[:, :], [128, 32])
            stepA_sb = load(cpool, "stepA", stepA[:, :], [128, 128], dt=BF)
            negB_sb = load(cpool, "negB", negB[:, :], [128, 128], dt=BF)
            ones_sb = load(cpool, "onesc", onesc[:, :], [128, 128], dt=BF)
            idn_sb = load(cpool, "idn", idn[:, :], [128, 128], dt=BF)
            bvp_sb = load(cpool, "bvp", bvp[:, :], [128, 16])
            wv_srcs = [wv0, wv1]
            wv_sb = []
            for d in range(2):
                t = bpool.tile([128, 2048], BF, tag=f"wv{d}", name=f"wv{d}")
                for ch in range(2):
                    nc.sync.dma_start(
                        out=t[:, ch * 1024:(ch + 1) * 1024],
                        in_=wv_srcs[d][:, ch * 1024:(ch + 1) * 1024],
                    )
                wv_sb.append(t)
            wot_sb = bpool.tile([128, 4096], BF, tag="wot", name="wot")
            for ch in range(2):
                nc.sync.dma_start(
                    out=wot_sb[:, ch * 2048:(ch + 1) * 2048],
                    in_=wot[:, ch * 2048:(ch + 1) * 2048],
                )

            # d-major Q^T/K^T/V^T, NATURAL order: cols = hl*1024 + s'
            qt = [bpool.tile([128, 4096], BF, tag=f"qt{c}", name=f"qt{c}")
                  for c in range(2)]
            kt = [bpool.tile([128, 4096], BF, tag=f"kt{c}", name=f"kt{c}")
                  for c in range(2)]
            vt = [bpool.tile([128, 4096], BF, tag=f"vt{c}", name=f"vt{c}")
                  for c in range(2)]
            # V k-major per head: [128 k, kb*256 + c*128 + d]
            v_nat = [bpool.tile([128, 2048], BF, tag=f"vn{hl}", name=f"vn{hl}")
                     for hl in range(HPC)]

            # ---- P1: Q^T/K^T projections (all heads), natural scatter ----
            pj_ctx = tc.tile_pool(name="ps_pj", bufs=4, space="PSUM")
            pjpool = pj_ctx.__enter__()
            for w in range(8):
                wps = pjpool.tile([128, 512], FDT, tag="pj", name="warm")
                mm(wps[:, :], wz[:, 0:128], wz[:, :], start=True, stop=True)
            nbias = 0
            for s in range(2):
                dst = qt if s == 0 else kt
                for cb in range(8):
                    for c in range(2):
                        ps = pjpool.tile([128, 512], FDT, tag="pj",
                                         name="proj")
                        for d in range(2):
                            mm(
                                ps[:, :],
                                wqk_sb[d][:, s * 2048 + cb * 256 + c * 128:
                                          s * 2048 + cb * 256 + c * 128 + 128],
                                xt_sb[d][:, :],
                                start=(d == 0), stop=(d == 1),
                            )
                        bi = s * 16 + cb * 2 + c
                        out_v = dst[c].rearrange(
                            "p (h r e) -> p h r e", h=4, r=128, e=8
                        )[:, :, :, cb]
                        in_v = ps.rearrange("p (h r) -> p h r", h=4)[:, :, :]
                        if nbias % 2 == 0:
                            nc.scalar.activation(
                                out_v, in_v, IDF, bias=bqk_sb[:, bi:bi + 1]
                            )
                        else:
                            nc.vector.tensor_scalar_add(
                                out=out_v, in0=in_v,
                                scalar1=bqk_sb[:, bi:bi + 1],
                            )
                        nbias += 1

            # ---- P2: V^T projection (all heads), natural scatter ----
            for cb in range(8):
                for c in range(2):
                    ps = pjpool.tile([128, 512], FDT, tag="pj", name="vproj")
                    for d in range(2):
                        mm(
                            ps[:, :],
                            wv_sb[d][:, cb * 256 + c * 128:
                                     cb * 256 + c * 128 + 128],
                            xt_sb[d][:, :],
                            start=(d == 0), stop=(d == 1),
                        )
                    bi = cb * 2 + c
                    out_v = vt[c].rearrange(
                        "p (h r e) -> p h r e", h=4, r=128, e=8
                    )[:, :, :, cb]
                    in_v = ps.rearrange("p (h r) -> p h r", h=4)[:, :, :]
                    if nbias % 2 == 0:
                        nc.scalar.activation(
                            out_v, in_v, IDF, bias=bvp_sb[:, bi:bi + 1]
                        )
                    else:
                        nc.vector.tensor_scalar_add(
                            out=out_v, in0=in_v, scalar1=bvp_sb[:, bi:bi + 1]
                        )
                    nbias += 1
            pj_ctx.__exit__(None, None, None)

            # ---- P2b: V -> k-major via PE transposes ----
            with tc.tile_pool(name="ps_t", bufs=2, space="PSUM") as tpool:
                ncp = 0
                for hl in range(HPC):
                    for kb in range(8):
                        tp = tpool.tile([128, 256], BF, tag="tp", name="tp")
                        for c in range(2):
                            nc.tensor.transpose(
                                tp[:, c * 128:(c + 1) * 128],
                                vt[c][:, hl * 1024 + kb * 128:
                                      hl * 1024 + (kb + 1) * 128],
                                idn_sb[:, :],
                            )
                        dst = v_nat[hl][:, kb * 256:(kb + 1) * 256]
                        if ncp % 2 == 0:
                            nc.scalar.copy(dst, tp[:, :])
                        else:
                            nc.vector.tensor_copy(out=dst, in_=tp[:, :])
                        ncp += 1

            # ---- P3: attention per head, causal block-skipped ----
            # pt strips per kb; PV/denominator as per-half [128,512] psum
            # accumulators with ONE wide matmul per (kb, half) — region qb
            # gets contributions from strips kb<=qb only, so per-mm stop
            # flags can't be exact: skip_group_check.
            with (
                tc.tile_pool(name="ps_se", bufs=2, space="PSUM") as pse,
                tc.tile_pool(name="ps_po", bufs=4, space="PSUM") as ppo,
            ):
                for hl in range(HPC):
                    qoff = hl * 1024
                    pt_h = ptpool.tile([128, 4608], BF, tag="pt", name="pt")
                    oh = [ohpool.tile([128, 1024], BF, tag="oh",
                                      name=f"oh{c}") for c in range(2)]

                    def strips(kbs, hl=hl, qoff=qoff, pt_h=pt_h):
                        for kb in kbs:
                            for chunk in CHUNKS[kb]:
                                n = len(chunk) * 128
                                sp = poolA.tile([128, 512], FDT, tag="pa",
                                                name="score")
                                if chunk[0] == kb:   # diagonal block first
                                    mm(sp[:, 0:128], stepA_sb[:, :],
                                       negB_sb[:, :], start=True, stop=False)
                                    for c in range(2):
                                        mm(sp[:, 0:128],
                                           kt[c][:, qoff + kb * 128:
                                                 qoff + kb * 128 + 128],
                                           qt[c][:, qoff + kb * 128:
                                                 qoff + kb * 128 + 128],
                                           start=False, stop=(c == 1))
                                    q0, w = 128, n - 128
                                else:
                                    q0, w = 0, n
                                if w > 0:
                                    qc0 = qoff + (chunk[0] * 128) + q0
                                    for c in range(2):
                                        mm(sp[:, q0:q0 + w],
                                           kt[c][:, qoff + kb * 128:
                                                 qoff + kb * 128 + 128],
                                           qt[c][:, qc0:qc0 + w],
                                           start=(c == 0), stop=(c == 1))
                                o0 = OFF[kb] + (chunk[0] - kb) * 128
                                nc.scalar.activation(
                                    pt_h[:, o0:o0 + n], sp[:, 0:n], EXP
                                )

                    def pv_half(half, hl=hl, pt_h=pt_h, oh=oh):
                        # cols [half*512, half*512+512) of the q axis
                        lo = half * 4
                        se = pse.tile([128, 512], FDT, tag="se", name="se")
                        pos = [ppo.tile([128, 512], FDT, tag="po",
                                        name=f"po{c}") for c in range(2)]
                        kmax = lo + 4
                        for c in range(-1, 2):
                            dst = se if c < 0 else pos[c]
                            for kb in range(kmax):
                                # q blocks covered: max(kb, lo)..lo+3
                                qb0 = max(kb, lo)
                                w = (lo + 4 - qb0) * 128
                                o = OFF[kb] + (qb0 - kb) * 128
                                r0 = (qb0 - lo) * 128
                                lhsT = ones_sb[:, :] if c < 0 else (
                                    v_nat[hl][:, kb * 256 + c * 128:
                                              kb * 256 + c * 128 + 128])
                                mm(dst[:, r0:r0 + w], lhsT,
                                   pt_h[:, o:o + w],
                                   start=(kb == 0), stop=(kb == kmax - 1),
                                   skip_group_check=True)
                        rc = rcpool.tile([128, 512], FDT, tag="rc", name="rc")
                        nc.vector.reciprocal(out=rc[:, :], in_=se[:, :])
                        for c in range(2):
                            nc.vector.tensor_mul(
                                out=oh[c][:, half * 512:half * 512 + 512],
                                in0=pos[c][:, :], in1=rc[:, :],
                            )

                    strips(range(0, 4))
                    pv_half(0)
                    strips(range(4, 8))
                    pv_half(1)

                    # ---- P4: output projection for this head ----
                    yp = pse.tile([128, 512], FDT, tag="se", name="yproj")
                    for cb in range(8):
                        for c in range(2):
                            j = cb * 2 + c
                            mm(
                                yp[:, 0:256],
                                oh[c].rearrange(
                                    "p (r e) -> p r e", r=128, e=8
                                )[:, :, cb],
                                wot_sb[:, j * 256:(j + 1) * 256],
                                start=(j == 0), stop=(j == 15),
                            )
                    ys = yspool.tile([128, 256], FDT, tag="ys", name="ys")
                    nc.scalar.copy(ys[:, :], yp[:, 0:256])
                    nc.sync.dma_start(
                        out=y[hl * 128:(hl + 1) * 128, :], in_=ys[:, :]
                    )
    nc.finalize()
    return nc


def _prep_inputs(x, w_attn, b_attn, w_out):
    b16 = ml_dtypes.bfloat16
    wqk = np.ascontiguousarray(
        np.concatenate([w_attn[0:2048] / 16.0, w_attn[2048:4096]]).T
    ).astype(b16)  # (256, 4096)
    wvt = np.ascontiguousarray(w_attn[4096:6144].T).astype(b16)  # (256, 2048)
    bqk_arr = np.ascontiguousarray(
        np.concatenate([b_attn[0:2048] / 16.0, b_attn[2048:4096]])
        .reshape(32, 128).T
    ).astype(np.float32)  # (128, 32)
    bvp_arr = np.ascontiguousarray(
        b_attn[4096:6144].reshape(16, 128).T
    ).astype(np.float32)  # (128, 16)
    wot_arr = np.ascontiguousarray(
        w_out.T.reshape(16, 128, 256).transpose(1, 0, 2).reshape(128, 4096)
    ).astype(b16)

    stepA = np.triu(np.ones((128, 128), np.float32)).astype(b16)
    negB = (NEG * np.eye(128, k=-1)).astype(b16)
    onesc = np.ones((128, 128), np.float32).astype(b16)
    idn = np.eye(128, dtype=np.float32).astype(b16)

    in_maps = []
    for cidx in range(NCORES):
        b, g = divmod(cidx, 2)
        xt = np.ascontiguousarray(
            x[b, 512 * g:512 * (g + 1)].T
        ).astype(b16)  # (256, 512)
        in_maps.append({
            "xt0": np.ascontiguousarray(xt[:128]),
            "xt1": np.ascontiguousarray(xt[128:]),
            "wqk0": np.ascontiguousarray(wqk[:128]),
            "wqk1": np.ascontiguousarray(wqk[128:]),
            "wv0": np.ascontiguousarray(wvt[:128]),
            "wv1": np.ascontiguousarray(wvt[128:]),
            "bqk": bqk_arr,
            "bvp": bvp_arr,
            "wot": wot_arr,
            "stepA": stepA,
            "negB": negB,
            "onesc": onesc,
            "idn": idn,
        })
    return in_maps


def kernel(x, w_attn, b_attn, w_out, b_out):
    x = np.asarray(x, dtype=np.float32)
    w_attn = np.asarray(w_attn, dtype=np.float32)
    b_attn = np.asarray(b_attn, dtype=np.float32)
    w_out = np.asarray(w_out, dtype=np.float32)
    b_out = np.asarray(b_out, dtype=np.float32)

    if "nc" not in _cache:
        _cache["nc"] = _build()
    nc = _cache["nc"]

    in_maps = _prep_inputs(x, w_attn, b_attn, w_out)
    res = run_bass_kernel_spmd(nc, in_maps, list(range(NCORES))).results

    out = np.empty((BS, SQL, EDIM), dtype=np.float32)
    for c in range(NCORES):
        b, g = divmod(c, 2)
        out[b, 512 * g:512 * (g + 1)] = res[c]["y"]
    out += b_out
    return out


# revision 17
# speedup vs baseline: 1.1935x; 1.0436x over previous
"""Masked multi-head attention on 8 NeuronCores (faithful torch raw-view semantics).

The reference reshapes (bs, sql, nh*edim) -> (bs, nh, sql, edim) as a RAW VIEW:
head h's length-1024 pseudo-sequence is built from x rows 128h..128h+127 (each
row contributes 8 pseudo-positions, one per 256-col block of the projection),
and output rows 128h..128h+128 depend only on head h. So the work splits into
32 independent (batch, head) pairs -> 4 per core, no cross-core reduction.

v2: NATURAL pseudo-position ordering (column u = s' = r*8 + cb, a stride-8
scatter at projection writeback) makes the causal mask block-triangular, so
only 36 of 64 score/PV 128x128 blocks per head are computed (the baseline's
permuted ordering made every block half-masked -> full 64). All attention
matmuls run in bf16 (1 cycle/row at 128-wide tiles). The in-block causal
triangle on diagonal blocks is injected INTO PSUM by one small matmul
(step-matrix @ shifted-NEG-diag) instead of a DVE mask add. exp runs on the
Act engine straight from PSUM in per-kb strips; softmax denominators come from
bf16 ones-matmuls accumulated per q-block; V is re-laid out k-major via PE
transposes. Q weights/bias pre-scaled by 1/16.
"""

import sys

sys.path.insert(0, "/opt/trn_rl_repo")

import ml_dtypes
import numpy as np

from concourse import bacc, mybir
from concourse.tile import TileContext
from concourse.bass_utils import run_bass_kernel_spmd

EDIM = 256
BS = 4
SQL = 1024
HPC = 4           # heads per core
NCORES = 8
FDT = mybir.dt.float32
BF = mybir.dt.bfloat16
NEG = -1.0e30

# strip kb covers q-blocks kb..7; OFF[kb] = col offset of strip kb in pt
OFF = [0]
for _kb in range(1, 8):
    OFF.append(OFF[-1] + (8 - _kb + 1) * 128)
# chunks of <=4 q-blocks per strip (PSUM bank = 512 fp32 cols)
CHUNKS = {kb: [list(range(kb, 8))[i:i + 4]
               for i in range(0, 8 - kb, 4)] for kb in range(8)}

_cache = {}


def _build():
    nc = bacc.Bacc(dynamic_dma_scratch_size=512)

    F8 = mybir.dt.float8e4
    xh = nc.declare_dram_parameter("xh", [128, 1024], F8, isOutput=False)
    xl = nc.declare_dram_parameter("xl", [128, 1024], F8, isOutput=False)
    wqkh = nc.declare_dram_parameter("wqkh", [128, 8192], F8, isOutput=False)
    wqkl = nc.declare_dram_parameter("wqkl", [128, 8192], F8, isOutput=False)
    wvh = nc.declare_dram_parameter("wvh", [128, 4096], F8, isOutput=False)
    wvl = nc.declare_dram_parameter("wvl", [128, 4096], F8, isOutput=False)
    bqk = nc.declare_dram_parameter("bqk", [128, 32], FDT, isOutput=False)
    bvp = nc.declare_dram_parameter("bvp", [128, 16], FDT, isOutput=False)
    wot = nc.declare_dram_parameter("wot", [128, 4096], BF, isOutput=False)
    stepA = nc.declare_dram_parameter("stepA", [128, 128], BF, isOutput=False)
    negB = nc.declare_dram_parameter("negB", [128, 128], BF, isOutput=False)
    onesc = nc.declare_dram_parameter("onesc", [128, 128], BF, isOutput=False)
    idn = nc.declare_dram_parameter("idn", [128, 128], BF, isOutput=False)
    y = nc.declare_dram_parameter("y", [512, 256], FDT, isOutput=True)

    EXP = mybir.ActivationFunctionType.Exp
    IDF = mybir.ActivationFunctionType.Identity

    with TileContext(nc) as tc:
        with (
            tc.tile_pool(name="const", bufs=1) as cpool,
            tc.tile_pool(name="big", bufs=1) as bpool,
            tc.tile_pool(name="pt", bufs=2) as ptpool,
            tc.tile_pool(name="oh", bufs=4) as ohpool,
            tc.tile_pool(name="rc", bufs=4) as rcpool,
            tc.tile_pool(name="ys", bufs=2) as yspool,
            tc.tile_pool(name="ps_a", bufs=2, space="PSUM") as poolA,
        ):
            def load(pool, name, src, shape, dt=FDT, tag=None):
                t = pool.tile(shape, dt, tag=tag or name, name=name)
                nc.sync.dma_start(out=t[:, :], in_=src)
                return t

            def mm(out, lhsT, rhs, **kw):
                nc.tensor.matmul(out, lhsT, rhs, **kw)

            # ---- PE warmup: keep PE busy from t=0 on a zeroed const so the
            # p-state ramp (3us to full clock) burns off before real work ----
            wz = cpool.tile([128, 512], BF, tag="wz", name="warmzero")
            nc.vector.memset(wz[:, :], 0.0)

            # ---- DMAs in consumption order ----
            # x on the Activation HWDGE queue, weights on SP: parallel issue
            xh_sb = cpool.tile([128, 1024], F8, tag="xh", name="xh")
            nc.scalar.dma_start(out=xh_sb[:, :], in_=xh[:, :])
            xl_sb = cpool.tile([128, 1024], F8, tag="xl", name="xl")
            nc.scalar.dma_start(out=xl_sb[:, :], in_=xl[:, :])
            wqkh_sb = bpool.tile([128, 8192], F8, tag="wqkh", name="wqkh")
            wqkl_sb = bpool.tile([128, 8192], F8, tag="wqkl", name="wqkl")
            for s in range(2):          # Q cols then K cols (both i-halves)
                for t, src in ((wqkh_sb, wqkh), (wqkl_sb, wqkl)):
                    for i in range(2):
                        c0 = i * 4096 + s * 2048
                        nc.sync.dma_start(
                            out=t[:, c0:c0 + 2048], in_=src[:, c0:c0 + 2048]
                        )
                if s == 0:
                    bqk_sb = load(cpool, "bqk", bqk[:, :], [128, 32])
            stepA_sb = load(cpool, "stepA", stepA[:, :], [128, 128], dt=BF)
            negB_sb = load(cpool, "negB", negB[:, :], [128, 128], dt=BF)
            ones_sb = load(cpool, "onesc", onesc[:, :], [128, 128], dt=BF)
            idn_sb = load(cpool, "idn", idn[:, :], [128, 128], dt=BF)
            bvp_sb = load(cpool, "bvp", bvp[:, :], [128, 16])
            wvh_sb = load(bpool, "wvh", wvh[:, :], [128, 4096], dt=F8)
            wvl_sb = load(bpool, "wvl", wvl[:, :], [128, 4096], dt=F8)
            wot_sb = bpool.tile([128, 4096], BF, tag="wot", name="wot")
            for ch in range(2):
                nc.sync.dma_start(
                    out=wot_sb[:, ch * 2048:(ch + 1) * 2048],
                    in_=wot[:, ch * 2048:(ch + 1) * 2048],
                )

            # d-major Q^T/K^T/V^T, NATURAL order: cols = hl*1024 + s'
            qt = [bpool.tile([128, 4096], BF, tag=f"qt{c}", name=f"qt{c}")
                  for c in range(2)]
            kt = [bpool.tile([128, 4096], BF, tag=f"kt{c}", name=f"kt{c}")
                  for c in range(2)]
            vt = [bpool.tile([128, 4096], BF, tag=f"vt{c}", name=f"vt{c}")
                  for c in range(2)]
            # V k-major per head: [128 k, kb*256 + c*128 + d]
            v_nat = [bpool.tile([128, 2048], BF, tag=f"vn{hl}", name=f"vn{hl}")
                     for hl in range(HPC)]

            # ---- P1: Q^T/K^T projections (all heads), natural scatter ----
            pj_ctx = tc.tile_pool(name="ps_pj", bufs=4, space="PSUM")
            pjpool = pj_ctx.__enter__()
            for w in range(8):
                wps = pjpool.tile([128, 512], FDT, tag="pj", name="warm")
                mm(wps[:, :], wz[:, 0:128], wz[:, :], start=True, stop=True)
            nbias = 0
            DR = mybir.MatmulPerfMode.DoubleRow
            xhv = xh_sb.rearrange("p (two n) -> p two n", two=2)[:, :, :]
            xlv = xl_sb.rearrange("p (two n) -> p two n", two=2)[:, :, :]

            def wview(t, j0):
                return t.rearrange("p (two j) -> p two j", two=2)[
                    :, :, j0:j0 + 128]

            def proj_dr(ps, th, tl, j0):
                # (w_hi + w_lo).T @ (x_hi + x_lo), dropping the lo*lo term
                mm(ps[:, :], wview(th, j0), xhv, start=True, stop=False,
                   perf_mode=DR)
                mm(ps[:, :], wview(tl, j0), xhv, start=False, stop=False,
                   perf_mode=DR)
                mm(ps[:, :], wview(th, j0), xlv, start=False, stop=True,
                   perf_mode=DR)

            for s in range(2):
                dst = qt if s == 0 else kt
                for cb in range(8):
                    for c in range(2):
                        ps = pjpool.tile([128, 512], FDT, tag="pj",
                                         name="proj")
                        proj_dr(ps, wqkh_sb, wqkl_sb,
                                s * 2048 + cb * 256 + c * 128)
                        bi = s * 16 + cb * 2 + c
                        out_v = dst[c].rearrange(
                            "p (h r e) -> p h r e", h=4, r=128, e=8
                        )[:, :, :, cb]
                        in_v = ps.rearrange("p (h r) -> p h r", h=4)[:, :, :]
                        if nbias % 2 == 0:
                            nc.scalar.activation(
                                out_v, in_v, IDF, scale=1.0 / 4096.0,
                                bias=bqk_sb[:, bi:bi + 1],
                            )
                        else:
                            nc.vector.tensor_scalar(
                                out=out_v, in0=in_v, scalar1=1.0 / 4096.0,
                                scalar2=bqk_sb[:, bi:bi + 1],
                                op0=mybir.AluOpType.mult,
                                op1=mybir.AluOpType.add,
                            )
                        nbias += 1

            # ---- P2: V^T projection (all heads), natural scatter ----
            for cb in range(8):
                for c in range(2):
                    ps = pjpool.tile([128, 512], FDT, tag="pj", name="vproj")
                    proj_dr(ps, wvh_sb, wvl_sb, cb * 256 + c * 128)
                    bi = cb * 2 + c
                    out_v = vt[c].rearrange(
                        "p (h r e) -> p h r e", h=4, r=128, e=8
                    )[:, :, :, cb]
                    in_v = ps.rearrange("p (h r) -> p h r", h=4)[:, :, :]
                    if nbias % 2 == 0:
                        nc.scalar.activation(
                            out_v, in_v, IDF, scale=1.0 / 4096.0,
                            bias=bvp_sb[:, bi:bi + 1],
                        )
                    else:
                        nc.vector.tensor_scalar(
                            out=out_v, in0=in_v, scalar1=1.0 / 4096.0,
                            scalar2=bvp_sb[:, bi:bi + 1],
                            op0=mybir.AluOpType.mult,
                            op1=mybir.AluOpType.add,
                        )
                    nbias += 1
            pj_ctx.__exit__(None, None, None)

            # ---- P2b: V -> k-major via PE transposes ----
            with tc.tile_pool(name="ps_t", bufs=2, space="PSUM") as tpool:
                ncp = 0
                for hl in range(HPC):
                    for kb in range(8):
                        tp = tpool.tile([128, 256], BF, tag="tp", name="tp")
                        for c in range(2):
                            nc.tensor.transpose(
                                tp[:, c * 128:(c + 1) * 128],
                                vt[c][:, hl * 1024 + kb * 128:
                                      hl * 1024 + (kb + 1) * 128],
                                idn_sb[:, :],
                            )
                        dst = v_nat[hl][:, kb * 256:(kb + 1) * 256]
                        if ncp % 2 == 0:
                            nc.scalar.copy(dst, tp[:, :])
                        else:
                            nc.vector.tensor_copy(out=dst, in_=tp[:, :])
                        ncp += 1

            # ---- P3: attention per head, causal block-skipped ----
            # pt strips per kb; PV/denominator as per-half [128,512] psum
            # accumulators with ONE wide matmul per (kb, half) — region qb
            # gets contributions from strips kb<=qb only, so per-mm stop
            # flags can't be exact: skip_group_check.
            with (
                tc.tile_pool(name="ps_se", bufs=2, space="PSUM") as pse,
                tc.tile_pool(name="ps_po", bufs=4, space="PSUM") as ppo,
            ):
                for hl in range(HPC):
                    qoff = hl * 1024
                    pt_h = ptpool.tile([128, 4608], BF, tag="pt", name="pt")
                    oh = [ohpool.tile([128, 1024], BF, tag="oh",
                                      name=f"oh{c}") for c in range(2)]

                    def strips(kbs, hl=hl, qoff=qoff, pt_h=pt_h):
                        for kb in kbs:
                            for chunk in CHUNKS[kb]:
                                n = len(chunk) * 128
                                sp = poolA.tile([128, 512], FDT, tag="pa",
                                                name="score")
                                if chunk[0] == kb:   # diagonal block first
                                    mm(sp[:, 0:128], stepA_sb[:, :],
                                       negB_sb[:, :], start=True, stop=False)
                                    for c in range(2):
                                        mm(sp[:, 0:128],
                                           kt[c][:, qoff + kb * 128:
                                                 qoff + kb * 128 + 128],
                                           qt[c][:, qoff + kb * 128:
                                                 qoff + kb * 128 + 128],
                                           start=False, stop=(c == 1))
                                    q0, w = 128, n - 128
                                else:
                                    q0, w = 0, n
                                if w > 0:
                                    qc0 = qoff + (chunk[0] * 128) + q0
                                    for c in range(2):
                                        mm(sp[:, q0:q0 + w],
                                           kt[c][:, qoff + kb * 128:
                                                 qoff + kb * 128 + 128],
                                           qt[c][:, qc0:qc0 + w],
                                           start=(c == 0), stop=(c == 1))
                                o0 = OFF[kb] + (chunk[0] - kb) * 128
                                nc.scalar.activation(
                                    pt_h[:, o0:o0 + n], sp[:, 0:n], EXP
                                )

                    def pv_half(half, hl=hl, pt_h=pt_h, oh=oh):
                        # cols [half*512, half*512+512) of the q axis
                        lo = half * 4
                        se = pse.tile([128, 512], FDT, tag="se", name="se")
                        pos = [ppo.tile([128, 512], FDT, tag="po",
                                        name=f"po{c}") for c in range(2)]
                        kmax = lo + 4
                        for c in range(-1, 2):
                            dst = se if c < 0 else pos[c]
                            for kb in range(kmax):
                                # q blocks covered: max(kb, lo)..lo+3
                                qb0 = max(kb, lo)
                                w = (lo + 4 - qb0) * 128
                                o = OFF[kb] + (qb0 - kb) * 128
                                r0 = (qb0 - lo) * 128
                                lhsT = ones_sb[:, :] if c < 0 else (
                                    v_nat[hl][:, kb * 256 + c * 128:
                                              kb * 256 + c * 128 + 128])
                                mm(dst[:, r0:r0 + w], lhsT,
                                   pt_h[:, o:o + w],
                                   start=(kb == 0), stop=(kb == kmax - 1),
                                   skip_group_check=True)
                        rc = rcpool.tile([128, 512], FDT, tag="rc", name="rc")
                        nc.vector.reciprocal(out=rc[:, :], in_=se[:, :])
                        for c in range(2):
                            nc.vector.tensor_mul(
                                out=oh[c][:, half * 512:half * 512 + 512],
                                in0=pos[c][:, :], in1=rc[:, :],
                            )

                    strips(range(0, 4))
                    pv_half(0)
                    strips(range(4, 8))
                    pv_half(1)

                    # ---- P4: output projection for this head ----
                    yp = pse.tile([128, 512], FDT, tag="se", name="yproj")
                    for cb in range(8):
                        for c in range(2):
                            j = cb * 2 + c
                            mm(
                                yp[:, 0:256],
                                oh[c].rearrange(
                                    "p (r e) -> p r e", r=128, e=8
                                )[:, :, cb],
                                wot_sb[:, j * 256:(j + 1) * 256],
                                start=(j == 0), stop=(j == 15),
                            )
                    ys = yspool.tile([128, 256], FDT, tag="ys", name="ys")
                    nc.scalar.copy(ys[:, :], yp[:, 0:256])
                    nc.sync.dma_start(
                        out=y[hl * 128:(hl + 1) * 128, :], in_=ys[:, :]
                    )
    nc.finalize()
    return nc


def _hilo_dr(a, scale):
    """(256, N) f32 -> fp8e4m3 hi/lo pair in DoubleRow layout [128, 2N]:
    out[p, i*N + n] = a[i*128 + p, n] * scale. Power-of-2 scale lifts the
    values out of e4m3's subnormal range; the kernel divides it back out
    in the bias-add."""
    f8 = ml_dtypes.float8_e4m3
    a = a * scale
    hi = a.astype(f8)
    lo = (a - hi.astype(np.float32)).astype(f8)
    n = a.shape[1]

    def pack(m):
        return np.ascontiguousarray(
            m.reshape(2, 128, n).transpose(1, 0, 2).reshape(128, 2 * n)
        )

    return pack(hi), pack(lo)


def _prep_inputs(x, w_attn, b_attn, w_out):
    b16 = ml_dtypes.bfloat16
    wqk = np.ascontiguousarray(
        np.concatenate([w_attn[0:2048] / 16.0, w_attn[2048:4096]]).T
    )  # (256, 4096)
    wvt = np.ascontiguousarray(w_attn[4096:6144].T)  # (256, 2048)
    wqk_h, wqk_l = _hilo_dr(wqk, 256.0)
    wv_h, wv_l = _hilo_dr(wvt, 256.0)
    bqk_arr = np.ascontiguousarray(
        np.concatenate([b_attn[0:2048] / 16.0, b_attn[2048:4096]])
        .reshape(32, 128).T
    ).astype(np.float32)  # (128, 32)
    bvp_arr = np.ascontiguousarray(
        b_attn[4096:6144].reshape(16, 128).T
    ).astype(np.float32)  # (128, 16)
    wot_arr = np.ascontiguousarray(
        w_out.T.reshape(16, 128, 256).transpose(1, 0, 2).reshape(128, 4096)
    ).astype(b16)

    stepA = np.triu(np.ones((128, 128), np.float32)).astype(b16)
    negB = (NEG * np.eye(128, k=-1)).astype(b16)
    onesc = np.ones((128, 128), np.float32).astype(b16)
    idn = np.eye(128, dtype=np.float32).astype(b16)

    in_maps = []
    for cidx in range(NCORES):
        b, g = divmod(cidx, 2)
        xt = np.ascontiguousarray(
            x[b, 512 * g:512 * (g + 1)].T
        )  # (256, 512)
        x_h, x_l = _hilo_dr(xt, 16.0)
        in_maps.append({
            "xh": x_h,
            "xl": x_l,
            "wqkh": wqk_h,
            "wqkl": wqk_l,
            "wvh": wv_h,
            "wvl": wv_l,
            "bqk": bqk_arr,
            "bvp": bvp_arr,
            "wot": wot_arr,
            "stepA": stepA,
            "negB": negB,
            "onesc": onesc,
            "idn": idn,
        })
    return in_maps


def kernel(x, w_attn, b_attn, w_out, b_out):
    x = np.asarray(x, dtype=np.float32)
    w_attn = np.asarray(w_attn, dtype=np.float32)
    b_attn = np.asarray(b_attn, dtype=np.float32)
    w_out = np.asarray(w_out, dtype=np.float32)
    b_out = np.asarray(b_out, dtype=np.float32)

    if "nc" not in _cache:
        _cache["nc"] = _build()
    nc = _cache["nc"]

    in_maps = _prep_inputs(x, w_attn, b_attn, w_out)
    res = run_bass_kernel_spmd(nc, in_maps, list(range(NCORES))).results

    out = np.empty((BS, SQL, EDIM), dtype=np.float32)
    for c in range(NCORES):
        b, g = divmod(c, 2)
        out[b, 512 * g:512 * (g + 1)] = res[c]["y"]
    out += b_out
    return out
